# revision 1
# baseline (speedup 1.0000x reference)
"""Contrastive loss (batch-hard triplet, within batch) on 8 Trainium2 cores.

Math (matches the jax reference):
    xn = x / ||x||_2 (rows)                      [B, C] = [4096, 1024]
    g[i,j] = xn_i . xn_j
    d[i,j] = max(2 - 2 g, 0)   (since ||xn||=1)
    pos_i  = sum_{j: same label, j != i} d[i,j]
    neg_i  = min_{j: diff label} d[i,j]
    loss   = mean(relu(pos_i + 0.5 - neg_i))

Sharding: rows (anchors) split 512/core; every core computes its
[512, 4096] tile of the distance matrix. The host passes x both ways
is unnecessary — only x^T (pure layout transform) plus per-core anchor
slices; ALL math (norms, normalize, matmul, reductions) runs on device.

Device pipeline, per j-slice s of 512 columns (1 anchor slice + 8 full):
  load xT chunks [128, 512] f32 -> square (ACT/DVE) -> PE ones-matmul
  column-reduce -> sq [1,512] -> sqrt -> 1/x -> PE broadcast to [128,512]
  -> DVE multiply (bf16 out) -> xnt_s [128, 8, 512]  (k-major, no xbar!)

Label-mask fusion: 64 one-hot rows scaled +8 (anchor side) and -8 (rhs)
are appended to the contraction, so the PE produces
    m[i,j] = g[i,j] - 64 * same[i,j]
in one accumulation group. Then per matmul output tile:
    pos_half = sum_j relu(-m - 63)     (one ACT op, accum_out)
    mx       = max_j m                 (one DVE reduce)
    loss_i   = relu(2*(pos_half - relu(1 - mx)) + 0.5)
Per-core output is sum(loss_i)/4096; the host adds the 8 partials.
"""

import sys

if "/opt/trn_rl_repo" not in sys.path:
    sys.path.insert(0, "/opt/trn_rl_repo")

from contextlib import ExitStack

import ml_dtypes
import numpy as np

import concourse.bass as bass
import concourse.tile as tile
from concourse import bacc, mybir
from concourse.bass_utils import run_bass_kernel_spmd

B = 4096          # batch rows
C = 1024          # features
NCORES = 8
BA = B // NCORES  # anchors per core = 512
P = 128
KC = C // P       # 8 feature chunks of 128
NB = 512          # j-slice width
NJ = B // NB      # 8 j slices
NM = BA // P      # 4 anchor blocks (M=128 each)
NLAB = 64

F32 = mybir.dt.float32
BF16 = mybir.dt.bfloat16
FP8 = mybir.dt.float8e4
AF = mybir.ActivationFunctionType
AX = mybir.AxisListType

# fp8 path: matmul operands are (16*xn) in fp8e4 with DoubleRow pairs, so the
# PSUM holds 256*(g - 64*same); one-hots are +-128; post-ops rescale by 1/256.
import os
USE_FP8 = os.environ.get("CONTRASTIVE_FP8", "0") == "1"
ALPHA = 128.0 if USE_FP8 else 8.0   # onehot scale; product = -64 * XSCALE^2
XSCALE = 16.0 if USE_FP8 else 1.0
XDT = FP8 if USE_FP8 else BF16
PSC = 1.0 / (XSCALE * XSCALE)       # PSUM -> m rescale


def build_kernel():
    nc = bacc.Bacc("TRN2", target_bir_lowering=False, debug=False,
                   num_devices=NCORES)
    xt_d = nc.dram_tensor("xT", (C, B), F32, kind="ExternalInput").ap()
    xat_d = nc.dram_tensor("xaT", (C, BA), F32, kind="ExternalInput").ap()
    ohp_d = nc.dram_tensor("ohp", (NLAB, BA), XDT, kind="ExternalInput").ap()
    ohn_d = nc.dram_tensor("ohn", (NLAB, B), XDT, kind="ExternalInput").ap()
    out_d = nc.dram_tensor("out", (1, 1), F32, kind="ExternalOutput").ap()

    with tile.TileContext(nc) as tc, ExitStack() as ctx:
        big = ctx.enter_context(tc.tile_pool(name="big", bufs=1))
        xload = ctx.enter_context(tc.tile_pool(name="xload", bufs=16))
        sqp = ctx.enter_context(tc.tile_pool(name="sqp", bufs=3))
        stats = ctx.enter_context(tc.tile_pool(name="stats", bufs=3))
        scratch = ctx.enter_context(tc.tile_pool(name="scratch", bufs=2))
        psum = ctx.enter_context(tc.tile_pool(name="psum", bufs=5, space="PSUM"))
        psum1 = ctx.enter_context(tc.tile_pool(name="psum1", bufs=1, space="PSUM"))
        small = ctx.enter_context(tc.tile_pool(name="small", bufs=1))

        # xnt_s[p, c, j] = XSCALE * xn[s*512 + j, c*128 + p], one tile per slice
        xnts = [big.tile([P, KC, NB], XDT, name=f"xnt{s}", tag=f"xnt{s}")
                for s in range(NJ)]
        xat = big.tile([P, KC, BA], XDT)
        ohp = big.tile([NLAB, BA], XDT)
        ohn = big.tile([NLAB, B], XDT)
        pos_all = big.tile([P, NM * NJ], F32)
        max_all = big.tile([P, NM * NJ], F32)
        ones = big.tile([P, 1], F32)
        ones1 = big.tile([1, P], F32)
        ones128 = big.tile([P, P], BF16)
        bneg63 = big.tile([P, 1], F32)
        bhalf = big.tile([P, 1], F32)

        nc.sync.dma_start(ohp[:], ohp_d)
        nc.sync.dma_start(ohn[:], ohn_d)
        nc.vector.memset(ones[:], 1.0)
        nc.vector.memset(ones1[:], 1.0)
        nc.vector.memset(ones128[:], 1.0)
        nc.vector.memset(bneg63[:], -63.0)
        nc.vector.memset(bhalf[:], 0.5)

        def prep_slice(s):
            """s = -1: anchor slice -> xat; else j-slice s -> xnts[s]."""
            if s < 0:
                srcs = [xat_d[c * P:(c + 1) * P, :] for c in range(KC)]
                dst = xat
                w = BA
            else:
                srcs = [xt_d[c * P:(c + 1) * P, s * NB:(s + 1) * NB]
                        for c in range(KC)]
                dst = xnts[s]
                w = NB
            lts = []
            sq_ps = psum1.tile([P, NB], F32, tag="sqps", name="sq_ps")
            for c in range(KC):
                lt = xload.tile([P, NB], F32, tag="lt", name="lt")
                nc.sync.dma_start(lt[:, :w], srcs[c])
                xsq = sqp.tile([P, NB], BF16, tag="xsq", name="xsq")
                if c < 5:
                    nc.scalar.square(xsq[:, :w], lt[:, :w])
                elif c < 7:
                    nc.gpsimd.tensor_mul(xsq[:, :w], lt[:, :w], lt[:, :w])
                else:
                    nc.vector.tensor_mul(xsq[:, :w], lt[:, :w], lt[:, :w])
                # M=128 ones weights: every out row = column-sums; row 0 used
                nc.tensor.matmul(sq_ps[:, :w], ones128[:], xsq[:, :w],
                                 start=(c == 0), stop=(c == KC - 1))
                lts.append(lt)
            nrm = stats.tile([1, NB], F32, tag="nrm", name="nrm")
            # nrm = sqrt(sq)/XSCALE, so inv = XSCALE/||x|| folds the fp8 scale
            nc.scalar.activation(nrm[:, :w], sq_ps[0:1, :w], AF.Sqrt,
                                 scale=PSC)
            bc_ps = psum1.tile([P, NB], F32, tag="bcps", name="bc_ps")
            nc.tensor.matmul(bc_ps[:, :w], ones1[:], nrm[:, :w],
                             start=True, stop=True)
            invb = scratch.tile([P, NB], F32, tag="invb", name="invb")
            nc.vector.reciprocal(invb[:, :w], bc_ps[:, :w])
            for c in range(KC):
                eng = nc.vector if c < 5 else nc.gpsimd
                eng.tensor_mul(dst[:, c, :w], lts[c][:, :w], invb[:, :w])

        prep_slice(-1)

        # ---- main: m = g - 64*same via augmented matmul; fused reductions ----
        for jb in range(NJ):
            prep_slice(jb)
            pts = [psum.tile([P, NB], F32, tag="pt", name="pt")
                   for _ in range(NM)]
            if USE_FP8:
                for cp in range(KC // 2 + 1):
                    for m in range(NM):
                        if cp < KC // 2:
                            lhsT = xat[:, 2 * cp:2 * cp + 2, m * P:(m + 1) * P]
                            rhs = xnts[jb][:, 2 * cp:2 * cp + 2, :]
                            pm = mybir.MatmulPerfMode.DoubleRow
                        else:
                            lhsT = ohp[:, m * P:(m + 1) * P]
                            rhs = ohn[:, jb * NB:(jb + 1) * NB]
                            pm = None
                        nc.tensor.matmul(pts[m][:], lhsT, rhs, perf_mode=pm,
                                         start=(cp == 0), stop=(cp == KC // 2))
            else:
                for c in range(KC + 1):
                    for m in range(NM):
                        if c < KC:
                            lhsT = xat[:, c, m * P:(m + 1) * P]
                            rhs = xnts[jb][:, c, :]
                        else:
                            lhsT = ohp[:, m * P:(m + 1) * P]
                            rhs = ohn[:, jb * NB:(jb + 1) * NB]
                        nc.tensor.matmul(pts[m][:], lhsT, rhs,
                                         start=(c == 0), stop=(c == KC))
            for m in range(NM):
                col = m * NJ + jb
                rld = scratch.tile([P, NB], F32, tag="rld", name="rld")
                nc.scalar.activation(rld[:], pts[m][:], AF.Relu,
                                     bias=bneg63[:], scale=-PSC,
                                     accum_out=pos_all[:, col:col + 1])
                nc.vector.reduce_max(max_all[:, col:col + 1], pts[m][:],
                                     axis=AX.X)

        # ---- tail: per-anchor loss, partition-sum, scale ----
        posg = small.tile([P, NM], F32)
        nc.vector.reduce_sum(posg[:], pos_all.rearrange("p (m j) -> p m j", j=NJ),
                             axis=AX.X)
        maxg = small.tile([P, NM], F32)
        nc.vector.reduce_max(maxg[:], max_all.rearrange("p (m j) -> p m j", j=NJ),
                             axis=AX.X)
        hneg = small.tile([P, NM], F32)
        nc.scalar.activation(hneg[:], maxg[:], AF.Relu, bias=1.0, scale=-PSC)
        diff = small.tile([P, NM], F32)
        nc.vector.tensor_sub(diff[:], posg[:], hneg[:])
        loss = small.tile([P, NM], F32)
        nc.scalar.activation(loss[:], diff[:], AF.Relu, bias=bhalf[:], scale=2.0)
        psc = psum1.tile([1, NM], F32, tag="psc")
        nc.tensor.matmul(psc[:], ones[:], loss[:], start=True, stop=True)
        red = small.tile([1, 1], F32)
        nc.vector.reduce_sum(red[:], psc[:], axis=AX.X)
        outt = small.tile([1, 1], F32)
        nc.scalar.mul(outt[:], red[:], 1.0 / B)
        nc.sync.dma_start(out_d, outt[:])

    nc.compile()
    return nc


_NC = None


def _get_nc():
    global _NC
    if _NC is None:
        _NC = build_kernel()
    return _NC


def make_in_maps(x, label):
    x = np.ascontiguousarray(np.asarray(x, dtype=np.float32))
    label = np.asarray(label).astype(np.int64)
    xT = np.ascontiguousarray(x.T)
    np_xdt = ml_dtypes.float8_e4m3 if USE_FP8 else ml_dtypes.bfloat16
    oh = np.zeros((NLAB, B), dtype=np.float32)
    oh[label, np.arange(B)] = 1.0
    ohp_full = (ALPHA * oh).astype(np_xdt)
    ohn_full = (-ALPHA * oh).astype(np_xdt)
    in_maps = []
    for c in range(NCORES):
        sl = slice(c * BA, (c + 1) * BA)
        in_maps.append({
            "xT": xT,
            "xaT": np.ascontiguousarray(xT[:, sl]),
            "ohp": np.ascontiguousarray(ohp_full[:, sl]),
            "ohn": ohn_full,
        })
    return in_maps


def kernel(x, label):
    nc = _get_nc()
    res = run_bass_kernel_spmd(nc, make_in_maps(x, label),
                               core_ids=list(range(NCORES)))
    total = sum(float(r["out"][0, 0]) for r in res.results)
    return np.float32(total)



# revision 8
# speedup vs baseline: 1.4641x; 1.4641x over previous
"""Contrastive loss (batch-hard triplet, within batch) on 8 Trainium2 cores.

Math (matches the jax reference):
    xn = x / ||x||_2 (rows)                      [B, C] = [4096, 1024]
    g[i,j] = xn_i . xn_j
    d[i,j] = max(2 - 2 g, 0)   (since ||xn||=1)
    pos_i  = sum_{j: same label, j != i} d[i,j]
    neg_i  = min_{j: diff label} d[i,j]
    loss   = mean(relu(pos_i + 0.5 - neg_i))

Sharding: rows (anchors) split 512/core. Each core gets xT COLUMN-ROTATED so
its own anchors are j-slice 0; the normalized slice 0 doubles as the matmul
stationary side (no separate anchor prep). All math (norms, normalize,
matmul, reductions) runs on device; host does layout only (transpose, roll,
one-hot build).

Device pipeline per j-slice s (512 cols): 8 chunk DMAs [128,512] f32 ->
squares (fp8, spread ACT/DVE/Pool) -> 4 PE DoubleRow ones-matmuls column-
reduce -> sq [1,512] -> sqrt (ACT) -> reciprocal_approx_fast (DVE, [1,512])
-> PE broadcast to [128,512] -> 8 muls -> xnt_s [128, 8, 512] fp8 (= 16*xn).

Label-mask fusion: 64 one-hot rows at +-128 (fp8) are appended to the
contraction so PSUM holds 256*(g - 64*same) in one accumulation group.
Main loop is restructured for PE weight reuse: for each j-slice group and
anchor block m, each stationary tile (fp8 DoubleRow pairs + one-hot) is
loaded ONCE via explicit ldweights and reused across the group's j-slices
(matmuls carry ldweights=False). Per output tile:
    pos_half = sum_j relu(-m/256 - 63)   (one ACT op, accum_out)
    mx       = max_j m                   (one DVE reduce)
    loss_i   = relu(2*(pos_half - relu(1 - mx/256)) + 0.5)
Per-core output is sum(loss_i)/4096; the host adds the 8 partials.
"""

import sys

if "/opt/trn_rl_repo" not in sys.path:
    sys.path.insert(0, "/opt/trn_rl_repo")

from contextlib import ExitStack

import ml_dtypes
import numpy as np

import concourse.bass as bass
import concourse.tile as tile
from concourse import bacc, mybir
from concourse.bass_utils import run_bass_kernel_spmd

B = 4096          # batch rows
C = 1024          # features
NCORES = 8
BA = B // NCORES  # anchors per core = 512
P = 128
KC = C // P       # 8 feature chunks of 128
NB = 512          # j-slice width
NJ = B // NB      # 8 j slices
NM = BA // P      # 4 anchor blocks (M=128 each)
NLAB = 64

F32 = mybir.dt.float32
BF16 = mybir.dt.bfloat16
FP8 = mybir.dt.float8e4
AF = mybir.ActivationFunctionType
AX = mybir.AxisListType
DR = mybir.MatmulPerfMode.DoubleRow

# matmul operands are (16*xn) in fp8e4 with DoubleRow pairs, so the PSUM
# holds 256*(g - 64*same); one-hots are +-128; post-ops rescale by 1/256.
ALPHA = 128.0
XSCALE = 16.0
PSC = 1.0 / (XSCALE * XSCALE)

# per-slice engine assignment for the 8 squares ('a'=ACT, 'v'=DVE, 'p'=Pool)
SQ_ENG = "aaaavvpp"
# per-slice engine assignment for the 8 normalize multiplies ('v'/'p')
MUL_ENG = "vvvvvppp"
# j-slice groups for the main loop (PSUM: len(group)*2 buffers in flight)
GROUPS = [(0, 1, 2), (3, 4, 5), (6, 7)]


def _mm_noload(nc, out, lhsT, rhs, **kw):
    inst = nc.tensor.matmul(out, lhsT, rhs, **kw)
    inst.ins.ldweights = False
    return inst


def build_kernel():
    nc = bacc.Bacc("TRN2", target_bir_lowering=False, debug=False,
                   num_devices=NCORES)
    xt_d = nc.dram_tensor("xT", (C, B), F32, kind="ExternalInput").ap()
    ohp_d = nc.dram_tensor("ohp", (NLAB, BA), FP8, kind="ExternalInput").ap()
    ohn_d = nc.dram_tensor("ohn", (NLAB, B), FP8, kind="ExternalInput").ap()
    out_d = nc.dram_tensor("out", (1, 1), F32, kind="ExternalOutput").ap()

    with tile.TileContext(nc) as tc, ExitStack() as ctx:
        big = ctx.enter_context(tc.tile_pool(name="big", bufs=1))
        xload = ctx.enter_context(tc.tile_pool(name="xload", bufs=32))
        sqp = ctx.enter_context(tc.tile_pool(name="sqp", bufs=3))
        stats = ctx.enter_context(tc.tile_pool(name="stats", bufs=4))
        rldp = ctx.enter_context(tc.tile_pool(name="rldp", bufs=4))
        scratch = ctx.enter_context(tc.tile_pool(name="scratch", bufs=2))
        psmain = ctx.enter_context(tc.tile_pool(name="psmain", bufs=6,
                                                space="PSUM"))
        pssq = ctx.enter_context(tc.tile_pool(name="pssq", bufs=1,
                                              space="PSUM"))
        psbc = ctx.enter_context(tc.tile_pool(name="psbc", bufs=1,
                                              space="PSUM"))
        small = ctx.enter_context(tc.tile_pool(name="small", bufs=1))

        # xnt_s[p, c, j] = XSCALE * xn[s*512 + j, c*128 + p]; slice 0 doubles
        # as the anchor (stationary) tile thanks to host rotation.
        xnts = [big.tile([P, KC, NB], FP8, name=f"xnt{s}", tag=f"xnt{s}")
                for s in range(NJ)]
        ohp = big.tile([NLAB, BA], FP8)
        ohn = big.tile([NLAB, B], FP8)
        pos_all = big.tile([P, NM * NJ], F32)
        max_all = big.tile([P, NM * NJ], F32)
        ones = big.tile([P, 1], F32)
        ones1 = big.tile([1, P], F32)
        ones2 = big.tile([P, 2, P], FP8)
        bneg63 = big.tile([P, 1], F32)
        bhalf = big.tile([P, 1], F32)

        nc.sync.dma_start(ohp[:], ohp_d)
        nc.sync.dma_start(ohn[:], ohn_d)
        nc.vector.memset(ones[:], 1.0)
        nc.vector.memset(ones1[:], 1.0)
        nc.vector.memset(ones2[:], 1.0)
        nc.vector.memset(bneg63[:], -63.0)
        nc.vector.memset(bhalf[:], 0.5)

        # issue every x chunk load up-front; 32 bufs = 4 slices in flight
        xins = []
        for s in range(NJ):
            row = []
            for c in range(KC):
                lt = xload.tile([P, NB], F32, tag="lt", name="lt")
                nc.sync.dma_start(
                    lt[:], xt_d[c * P:(c + 1) * P, s * NB:(s + 1) * NB])
                row.append(lt)
            xins.append(row)

        def prep_slice(s):
            lts = xins[s]
            xsq = sqp.tile([P, KC, NB], FP8, tag="xsq", name="xsq")
            for c in range(KC):
                e = SQ_ENG[c]
                if e == "a":
                    nc.scalar.square(xsq[:, c, :], lts[c][:])
                elif e == "v":
                    nc.vector.tensor_mul(xsq[:, c, :], lts[c][:], lts[c][:])
                else:
                    nc.gpsimd.tensor_mul(xsq[:, c, :], lts[c][:], lts[c][:])
            sq_ps = pssq.tile([P, NB], F32, tag="sqps", name="sq_ps")
            for g in range(KC // 2):
                inst = nc.tensor.matmul(sq_ps[:], ones2[:],
                                        xsq[:, 2 * g:2 * g + 2, :],
                                        perf_mode=DR, start=(g == 0),
                                        stop=(g == KC // 2 - 1))
                if g > 0:
                    inst.ins.ldweights = False
            # nrm = sqrt(sq)/XSCALE, so inv = XSCALE/||x|| folds the fp8 scale
            nrm = stats.tile([1, NB], F32, tag="nrm", name="nrm")
            nc.scalar.activation(nrm[:], sq_ps[0:1, :], AF.Sqrt, scale=PSC)
            inv = stats.tile([1, NB], F32, tag="inv", name="inv")
            nc.vector.reciprocal_approx_fast(inv[:], nrm[:])
            bc_ps = psbc.tile([P, NB], F32, tag="bcps", name="bc_ps")
            nc.tensor.matmul(bc_ps[:], ones1[:], inv[:], start=True, stop=True)
            invb = scratch.tile([P, NB], F32, tag="invb", name="invb")
            nc.scalar.copy(invb[:], bc_ps[:])
            for c in range(KC):
                eng = nc.vector if MUL_ENG[c] == "v" else nc.gpsimd
                eng.tensor_mul(xnts[s][:, c, :], lts[c][:], invb[:])

        def main_group(grp):
            for m in range(NM):
                pts = {}
                for jb in grp:
                    pts[jb] = psmain.tile([P, NB], F32, tag="pt", name="pt")
                for cg in range(KC // 2 + 1):
                    if cg < KC // 2:
                        w = xnts[0][:, 2 * cg:2 * cg + 2, m * P:(m + 1) * P]
                        pm = DR
                    else:
                        w = ohp[:, m * P:(m + 1) * P]
                        pm = None
                    for idx, jb in enumerate(grp):
                        if cg < KC // 2:
                            rhs = xnts[jb][:, 2 * cg:2 * cg + 2, :]
                        else:
                            rhs = ohn[:, jb * NB:(jb + 1) * NB]
                        inst = nc.tensor.matmul(pts[jb][:], w, rhs,
                                                perf_mode=pm,
                                                start=(cg == 0),
                                                stop=(cg == KC // 2))
                        if idx > 0:
                            inst.ins.ldweights = False
                for jb in grp:
                    col = m * NJ + jb
                    rld = rldp.tile([P, NB], BF16, tag="rld", name="rld")
                    nc.scalar.activation(rld[:], pts[jb][:], AF.Relu,
                                         bias=bneg63[:], scale=-PSC,
                                         accum_out=pos_all[:, col:col + 1])
                    nc.vector.reduce_max(max_all[:, col:col + 1], pts[jb][:],
                                         axis=AX.X)

        done = 0
        for s in range(NJ):
            prep_slice(s)
            for grp in GROUPS:
                if max(grp) == s:
                    main_group(grp)

        # ---- tail: per-anchor loss, partition-sum, scale ----
        posg = small.tile([P, NM], F32)
        nc.vector.reduce_sum(posg[:], pos_all.rearrange("p (m j) -> p m j", j=NJ),
                             axis=AX.X)
        maxg = small.tile([P, NM], F32)
        nc.vector.reduce_max(maxg[:], max_all.rearrange("p (m j) -> p m j", j=NJ),
                             axis=AX.X)
        hneg = small.tile([P, NM], F32)
        nc.scalar.activation(hneg[:], maxg[:], AF.Relu, bias=1.0, scale=-PSC)
        diff = small.tile([P, NM], F32)
        nc.vector.tensor_sub(diff[:], posg[:], hneg[:])
        loss = small.tile([P, NM], F32)
        nc.scalar.activation(loss[:], diff[:], AF.Relu, bias=bhalf[:], scale=2.0)
        psc = psmain.tile([1, NM], F32, tag="pt", name="pt")
        nc.tensor.matmul(psc[:], ones[:], loss[:], start=True, stop=True)
        red = small.tile([1, 1], F32)
        nc.vector.reduce_sum(red[:], psc[:], axis=AX.X)
        outt = small.tile([1, 1], F32)
        nc.scalar.mul(outt[:], red[:], 1.0 / B)
        nc.sync.dma_start(out_d, outt[:])

    nc.compile()
    return nc


_NC = None


def _get_nc():
    global _NC
    if _NC is None:
        _NC = build_kernel()
    return _NC


def make_in_maps(x, label):
    x = np.ascontiguousarray(np.asarray(x, dtype=np.float32))
    label = np.asarray(label).astype(np.int64)
    xT = np.ascontiguousarray(x.T)
    oh = np.zeros((NLAB, B), dtype=np.float32)
    oh[label, np.arange(B)] = 1.0
    in_maps = []
    for c in range(NCORES):
        sl = slice(c * BA, (c + 1) * BA)
        roll = -c * BA
        in_maps.append({
            "xT": np.ascontiguousarray(np.roll(xT, roll, axis=1)),
            "ohp": np.ascontiguousarray(
                (ALPHA * oh[:, sl]).astype(ml_dtypes.float8_e4m3)),
            "ohn": np.ascontiguousarray(
                np.roll(-ALPHA * oh, roll, axis=1).astype(ml_dtypes.float8_e4m3)),
        })
    return in_maps


def kernel(x, label):
    nc = _get_nc()
    res = run_bass_kernel_spmd(nc, make_in_maps(x, label),
                               core_ids=list(range(NCORES)))
    total = sum(float(r["out"][0, 0]) for r in res.results)
    return np.float32(total)
